# revision 1
# baseline (speedup 1.0000x reference)
"""Trainium2 Bass kernel for BaseNeuron degree-feature spiking forward.

Computes, for dv [500000, 128] f32, binned_degree [500000] i32,
v_threshold [20, 128] f32, tau, alpha scalars:

    v      = dv / tau
    thresh = v_threshold[binned_degree]          # per-node row gather
    spike  = (v - thresh > 0).astype(f32)        # heaviside forward
    sums   = segment_sum(spike, binned_degree)   # [20, 128]
    counts = bincount(binned_degree)             # [20]
    v_th_new = where(counts>0, 0.2*sums/max(counts,1) + 0.8*v_threshold,
                     v_threshold)

Returns (spike, v_th_new) like the reference.

Strategy: shard nodes across 8 NeuronCores (62500 nodes each, padded to
63488 = 496 tiles of 128). Per core the device computes spike and the
per-bin partial sums; the tiny [20,128] reduction across cores + the EMA
update run on host.

Device kernel per 4-tile group (512 nodes):
  - one matmul  thresh[128,512] = onehotT_stack[80,128].T @ blockdiag(vth)[80,512]
    (the one-hot matrices are precomputed on host from binned_degree)
  - one fused VectorE op  spike = (dv * (1/tau)) > thresh  (reads PSUM)
  - per 128-node tile, matmul  sums[20,128] += onehot[128,20].T @ spike[128,128]
    accumulated in a persistent PSUM bank over all 496 tiles.
"""

import os
import numpy as np

import concourse.bass as bass
import concourse.bacc as bacc
import concourse.mybir as mybir
from concourse.tile import TileContext
from concourse import bass_utils

F32 = mybir.dt.float32
ALU = mybir.AluOpType

# Problem geometry (hardcoded per contract).
N_FULL = 500000
S = 128
BINS = 20
CORES = 8
SHARD = N_FULL // CORES          # 62500 real nodes per core
TILES = 496                      # node tiles of 128 per core (padded)
PAD_ROWS = TILES * 128           # 63488 rows fed to each core
GROUP = 4                        # tiles per gather matmul (N = 512)
NGROUPS = TILES // GROUP         # 124
CHUNK = 16                       # tiles per DMA chunk (1 MiB)
NCHUNKS = TILES // CHUNK         # 31
GROUPS_PER_CHUNK = CHUNK // GROUP
GAMMA = np.float32(0.2)

_PROGRAM_CACHE = {}
LAST_RESULTS = None              # BassKernelResults of the most recent run


def _build_program(inv_tau: float):
    nc = bacc.Bacc(
        "TRN2",
        target_bir_lowering=False,
        debug=False,
        num_devices=CORES,
    )

    dv = nc.dram_tensor("dv", [PAD_ROWS, S], F32, kind="ExternalInput").ap()
    oh = nc.dram_tensor("oh", [128, TILES * BINS], F32, kind="ExternalInput").ap()
    ohT = nc.dram_tensor(
        "ohT", [GROUP * BINS, NGROUPS * 128], F32, kind="ExternalInput"
    ).ap()
    vth4 = nc.dram_tensor(
        "vth4", [GROUP * BINS, GROUP * S], F32, kind="ExternalInput"
    ).ap()
    spike = nc.dram_tensor("spike", [PAD_ROWS, S], F32, kind="ExternalOutput").ap()
    sums = nc.dram_tensor("sums", [BINS, S], F32, kind="ExternalOutput").ap()

    # [p, t, s] views: node row t*128+p  ->  partition p, free (t, s)
    dv_v = dv.rearrange("(t p) s -> p t s", p=128)
    spike_v = spike.rearrange("(t p) s -> p t s", p=128)

    with TileContext(nc) as tc:
        with (
            tc.tile_pool(name="const", bufs=1) as cpool,
            tc.tile_pool(name="io", bufs=3) as iopool,
            tc.tile_pool(name="psum", bufs=4, space="PSUM") as ppool,
            tc.tile_pool(name="acc", bufs=1, space="PSUM") as apool,
        ):
            oh_sb = cpool.tile([128, TILES * BINS], F32)
            nc.sync.dma_start(out=oh_sb[:], in_=oh)
            ohT_sb = cpool.tile([GROUP * BINS, NGROUPS * 128], F32)
            nc.sync.dma_start(out=ohT_sb[:], in_=ohT)
            vth4_sb = cpool.tile([GROUP * BINS, GROUP * S], F32)
            nc.sync.dma_start(out=vth4_sb[:], in_=vth4)

            sums_ps = apool.tile([BINS, S], F32)

            for c in range(NCHUNKS):
                dv_sb = iopool.tile([128, CHUNK * S], F32, tag="dv")
                nc.sync.dma_start(
                    out=dv_sb[:], in_=dv_v[:, c * CHUNK : (c + 1) * CHUNK, :]
                )
                spike_sb = iopool.tile([128, CHUNK * S], F32, tag="spike")

                for g in range(GROUPS_PER_CHUNK):
                    gg = c * GROUPS_PER_CHUNK + g        # global group index
                    th_ps = ppool.tile([128, GROUP * S], F32, tag="th")
                    nc.tensor.matmul(
                        th_ps[:],
                        ohT_sb[:, gg * 128 : (gg + 1) * 128],
                        vth4_sb[:],
                        start=True,
                        stop=True,
                    )
                    lo, hi = g * GROUP * S, (g + 1) * GROUP * S
                    nc.vector.scalar_tensor_tensor(
                        out=spike_sb[:, lo:hi],
                        in0=dv_sb[:, lo:hi],
                        scalar=float(inv_tau),
                        in1=th_ps[:],
                        op0=ALU.mult,
                        op1=ALU.is_gt,
                    )
                    for t in range(GROUP):
                        gt = gg * GROUP + t              # global tile index
                        nc.tensor.matmul(
                            sums_ps[:],
                            oh_sb[:, gt * BINS : (gt + 1) * BINS],
                            spike_sb[:, (g * GROUP + t) * S : (g * GROUP + t + 1) * S],
                            start=(gt == 0),
                            stop=(gt == TILES - 1),
                            skip_group_check=True,
                        )

                nc.sync.dma_start(
                    out=spike_v[:, c * CHUNK : (c + 1) * CHUNK, :], in_=spike_sb[:]
                )

            sums_sb = cpool.tile([BINS, S], F32)
            nc.scalar.copy(out=sums_sb[:], in_=sums_ps[:])
            nc.sync.dma_start(out=sums, in_=sums_sb[:])

    nc.compile()
    return nc


def _host_inputs(dv, binned_degree, v_threshold):
    """Build the 8 per-core input maps (shards + precomputed one-hots)."""
    dv = np.ascontiguousarray(dv, dtype=np.float32)
    bins = np.asarray(binned_degree).astype(np.int64)
    vth = np.asarray(v_threshold, dtype=np.float32)

    vth4 = np.zeros((GROUP * BINS, GROUP * S), dtype=np.float32)
    for t in range(GROUP):
        vth4[t * BINS : (t + 1) * BINS, t * S : (t + 1) * S] = vth

    in_maps = []
    for c in range(CORES):
        start = c * SHARD
        stop = start + PAD_ROWS
        if stop <= N_FULL:
            dv_shard = dv[start:stop]                    # contiguous view
        else:
            dv_shard = np.zeros((PAD_ROWS, S), dtype=np.float32)
            dv_shard[: N_FULL - start] = dv[start:]
        b = np.full(PAD_ROWS, -1, dtype=np.int64)
        avail = min(N_FULL - start, PAD_ROWS)
        b[:SHARD] = bins[start : start + SHARD]          # only own rows counted
        del avail
        binT = b.reshape(TILES, 128).T                   # [128, TILES]
        onehot = binT[:, :, None] == np.arange(BINS)     # [128, TILES, 20] bool
        oh = np.ascontiguousarray(
            onehot.reshape(128, TILES * BINS).astype(np.float32)
        )
        ohT = np.ascontiguousarray(
            onehot.reshape(128, NGROUPS, GROUP, BINS)
            .transpose(2, 3, 1, 0)                       # [GROUP, BINS, ngrp, 128]
            .reshape(GROUP * BINS, NGROUPS * 128)
            .astype(np.float32)
        )
        in_maps.append({"dv": dv_shard, "oh": oh, "ohT": ohT, "vth4": vth4})
    return in_maps


def kernel(dv, binned_degree, v_threshold, tau, alpha):
    global LAST_RESULTS
    inv_tau = 1.0 / float(tau)

    key = inv_tau
    if key not in _PROGRAM_CACHE:
        _PROGRAM_CACHE[key] = _build_program(inv_tau)
    nc = _PROGRAM_CACHE[key]

    in_maps = _host_inputs(dv, binned_degree, v_threshold)

    trace = bool(int(os.environ.get("KERNEL_TRACE", "0")))
    res = bass_utils.run_bass_kernel_spmd(
        nc, in_maps, core_ids=list(range(CORES)), trace=trace
    )
    LAST_RESULTS = res

    spike = np.concatenate(
        [res.results[c]["spike"][:SHARD] for c in range(CORES)], axis=0
    )

    sums = np.zeros((BINS, S), dtype=np.float32)
    for c in range(CORES):
        sums += res.results[c]["sums"]

    bins = np.asarray(binned_degree).astype(np.int64)
    counts = np.bincount(bins, minlength=BINS).astype(np.float32)
    vth = np.asarray(v_threshold, dtype=np.float32)
    mean = sums / np.maximum(counts, np.float32(1.0))[:, None]
    v_th_new = np.where(
        (counts > 0)[:, None],
        GAMMA * mean + (np.float32(1.0) - GAMMA) * vth,
        vth,
    ).astype(np.float32)

    return spike, v_th_new


# revision 2
# speedup vs baseline: 1.7176x; 1.7176x over previous
"""Trainium2 Bass kernel for BaseNeuron degree-feature spiking forward.

Computes, for dv [500000, 128] f32, binned_degree [500000] i32,
v_threshold [20, 128] f32, tau, alpha scalars:

    v      = dv / tau
    thresh = v_threshold[binned_degree]          # per-node row gather
    spike  = (v - thresh > 0).astype(f32)        # heaviside forward
    sums   = segment_sum(spike, binned_degree)   # [20, 128]
    counts = bincount(binned_degree)             # [20]
    v_th_new = where(counts>0, 0.2*sums/max(counts,1) + 0.8*v_threshold,
                     v_threshold)

Returns (spike, v_th_new) like the reference.

Strategy: shard nodes across 8 NeuronCores (62500 nodes each, padded to
63488 = 496 tiles of 128). Per core the device computes spike and the
per-bin partial sums; the tiny [20,128] reduction across cores + the EMA
update run on host.

Device kernel per 4-tile group (512 nodes):
  - one matmul  thresh[128,512] = onehotT_stack[80,128].T @ blockdiag(vth)[80,512]
    (one-hot matrices precomputed on host from binned_degree, bf16)
  - one fused VectorE op  spike_bf16 = (dv * (1/tau)) > thresh  (reads PSUM)
  - per 128-node tile, matmul  sums[20,128] += onehot[128,20].T @ spike[128,128]
    accumulated in a persistent PSUM bank over all 496 tiles.

dv and spike live in DRAM pre-permuted as [128, TILES*128] (partition-
contiguous rows -> large DMA descriptors); the host does the permutes.
spike travels as bf16 (exact for 0/1) and is widened to f32 on host.
"""

import os
import numpy as np
import ml_dtypes

import concourse.bass as bass
import concourse.bacc as bacc
import concourse.mybir as mybir
from concourse.tile import TileContext
from concourse import bass_utils

F32 = mybir.dt.float32
BF16 = mybir.dt.bfloat16
ALU = mybir.AluOpType
NP_BF16 = ml_dtypes.bfloat16

# Problem geometry (hardcoded per contract).
N_FULL = 500000
S = 128
BINS = 20
CORES = 8
SHARD = N_FULL // CORES          # 62500 real nodes per core
TILES = 496                      # node tiles of 128 per core (padded)
PAD_ROWS = TILES * 128           # 63488 rows fed to each core
GROUP = 4                        # tiles per gather matmul (N = 512)
NGROUPS = TILES // GROUP         # 124
CHUNK = 16                       # tiles per DMA chunk (1 MiB of dv)
NCHUNKS = TILES // CHUNK         # 31
GROUPS_PER_CHUNK = CHUNK // GROUP
GAMMA = np.float32(0.2)

_PROGRAM_CACHE = {}
LAST_RESULTS = None              # BassKernelResults of the most recent run


def _build_program(inv_tau: float, gather_f32: bool):
    GDT = F32 if gather_f32 else BF16
    nc = bacc.Bacc(
        "TRN2",
        target_bir_lowering=False,
        debug=False,
        num_devices=CORES,
    )

    dv = nc.dram_tensor("dv", [128, TILES * S], F32, kind="ExternalInput").ap()
    oh = nc.dram_tensor("oh", [128, TILES * BINS], BF16, kind="ExternalInput").ap()
    ohT = nc.dram_tensor(
        "ohT", [GROUP * BINS, NGROUPS * 128], GDT, kind="ExternalInput"
    ).ap()
    vth4 = nc.dram_tensor(
        "vth4", [GROUP * BINS, GROUP * S], GDT, kind="ExternalInput"
    ).ap()
    spike = nc.dram_tensor(
        "spike", [128, TILES * S], BF16, kind="ExternalOutput"
    ).ap()
    sums = nc.dram_tensor("sums", [BINS, S], F32, kind="ExternalOutput").ap()

    with TileContext(nc) as tc:
        with (
            tc.tile_pool(name="const", bufs=1) as cpool,
            tc.tile_pool(name="io", bufs=3) as iopool,
            tc.tile_pool(name="psum", bufs=4, space="PSUM") as ppool,
            tc.tile_pool(name="acc", bufs=1, space="PSUM") as apool,
        ):
            oh_sb = cpool.tile([128, TILES * BINS], BF16)
            nc.sync.dma_start(out=oh_sb[:], in_=oh)
            ohT_sb = cpool.tile([GROUP * BINS, NGROUPS * 128], GDT)
            nc.sync.dma_start(out=ohT_sb[:], in_=ohT)
            vth4_sb = cpool.tile([GROUP * BINS, GROUP * S], GDT)
            nc.sync.dma_start(out=vth4_sb[:], in_=vth4)

            sums_ps = apool.tile([BINS, S], F32)

            for c in range(NCHUNKS):
                dv_sb = iopool.tile([128, CHUNK * S], F32, tag="dv")
                nc.sync.dma_start(
                    out=dv_sb[:], in_=dv[:, c * CHUNK * S : (c + 1) * CHUNK * S]
                )
                spike_sb = iopool.tile([128, CHUNK * S], BF16, tag="spike")

                for g in range(GROUPS_PER_CHUNK):
                    gg = c * GROUPS_PER_CHUNK + g        # global group index
                    th_ps = ppool.tile([128, GROUP * S], F32, tag="th")
                    nc.tensor.matmul(
                        th_ps[:],
                        ohT_sb[:, gg * 128 : (gg + 1) * 128],
                        vth4_sb[:],
                        start=True,
                        stop=True,
                    )
                    lo, hi = g * GROUP * S, (g + 1) * GROUP * S
                    nc.vector.scalar_tensor_tensor(
                        out=spike_sb[:, lo:hi],
                        in0=dv_sb[:, lo:hi],
                        scalar=float(inv_tau),
                        in1=th_ps[:],
                        op0=ALU.mult,
                        op1=ALU.is_gt,
                    )
                    for t in range(GROUP):
                        gt = gg * GROUP + t              # global tile index
                        nc.tensor.matmul(
                            sums_ps[:],
                            oh_sb[:, gt * BINS : (gt + 1) * BINS],
                            spike_sb[:, (g * GROUP + t) * S : (g * GROUP + t + 1) * S],
                            start=(gt == 0),
                            stop=(gt == TILES - 1),
                            skip_group_check=True,
                        )

                nc.scalar.dma_start(
                    out=spike[:, c * CHUNK * S : (c + 1) * CHUNK * S],
                    in_=spike_sb[:],
                )

            sums_sb = cpool.tile([BINS, S], F32)
            nc.scalar.copy(out=sums_sb[:], in_=sums_ps[:])
            nc.sync.dma_start(out=sums, in_=sums_sb[:])

    nc.compile()
    return nc


def _host_inputs(dv, binned_degree, v_threshold, gather_f32: bool):
    """Build the 8 per-core input maps (permuted shards + one-hots)."""
    dv = np.ascontiguousarray(dv, dtype=np.float32)
    bins = np.asarray(binned_degree).astype(np.int64)
    vth = np.asarray(v_threshold, dtype=np.float32)
    gdt = np.float32 if gather_f32 else NP_BF16

    vth4 = np.zeros((GROUP * BINS, GROUP * S), dtype=np.float32)
    for t in range(GROUP):
        vth4[t * BINS : (t + 1) * BINS, t * S : (t + 1) * S] = vth
    vth4 = vth4.astype(gdt)

    in_maps = []
    for c in range(CORES):
        start = c * SHARD
        stop = start + PAD_ROWS
        if stop <= N_FULL:
            shard = dv[start:stop]
        else:
            shard = np.zeros((PAD_ROWS, S), dtype=np.float32)
            shard[: N_FULL - start] = dv[start:]
        # permute to [partition, tile*S]: dv2[p, t*S+s] = shard[t*128+p, s]
        dv2 = np.ascontiguousarray(
            shard.reshape(TILES, 128, S).transpose(1, 0, 2)
        ).reshape(128, TILES * S)

        b = np.full(PAD_ROWS, -1, dtype=np.int64)
        b[:SHARD] = bins[start : start + SHARD]          # only own rows counted
        binT = b.reshape(TILES, 128).T                   # [128, TILES]
        onehot = binT[:, :, None] == np.arange(BINS)     # [128, TILES, 20] bool
        oh = np.ascontiguousarray(
            onehot.reshape(128, TILES * BINS).astype(NP_BF16)
        )
        ohT = np.ascontiguousarray(
            onehot.reshape(128, NGROUPS, GROUP, BINS)
            .transpose(2, 3, 1, 0)                       # [GROUP, BINS, ngrp, 128]
            .reshape(GROUP * BINS, NGROUPS * 128)
            .astype(gdt)
        )
        in_maps.append({"dv": dv2, "oh": oh, "ohT": ohT, "vth4": vth4})
    return in_maps


def kernel(dv, binned_degree, v_threshold, tau, alpha):
    global LAST_RESULTS
    inv_tau = 1.0 / float(tau)

    vth = np.asarray(v_threshold, dtype=np.float32)
    gather_f32 = bool(
        (vth.astype(NP_BF16).astype(np.float32) != vth).any()
    )

    key = (inv_tau, gather_f32)
    if key not in _PROGRAM_CACHE:
        _PROGRAM_CACHE[key] = _build_program(inv_tau, gather_f32)
    nc = _PROGRAM_CACHE[key]

    in_maps = _host_inputs(dv, binned_degree, v_threshold, gather_f32)

    trace = bool(int(os.environ.get("KERNEL_TRACE", "0")))
    res = bass_utils.run_bass_kernel_spmd(
        nc, in_maps, core_ids=list(range(CORES)), trace=trace
    )
    LAST_RESULTS = res

    spike = np.empty((N_FULL, S), dtype=np.float32)
    for c in range(CORES):
        sp2 = res.results[c]["spike"]                    # [128, TILES*S] bf16
        sp = sp2.reshape(128, TILES, S).transpose(1, 0, 2).reshape(PAD_ROWS, S)
        spike[c * SHARD : (c + 1) * SHARD] = sp[:SHARD]  # widen bf16 -> f32

    sums = np.zeros((BINS, S), dtype=np.float32)
    for c in range(CORES):
        sums += res.results[c]["sums"]

    bins = np.asarray(binned_degree).astype(np.int64)
    counts = np.bincount(bins, minlength=BINS).astype(np.float32)
    mean = sums / np.maximum(counts, np.float32(1.0))[:, None]
    v_th_new = np.where(
        (counts > 0)[:, None],
        GAMMA * mean + (np.float32(1.0) - GAMMA) * vth,
        vth,
    ).astype(np.float32)

    return spike, v_th_new


# revision 3
# speedup vs baseline: 2.2404x; 1.3044x over previous
"""Trainium2 Bass kernel for BaseNeuron degree-feature spiking forward.

Computes, for dv [500000, 128] f32, binned_degree [500000] i32,
v_threshold [20, 128] f32, tau, alpha scalars:

    v      = dv / tau
    thresh = v_threshold[binned_degree]          # per-node row gather
    spike  = (v - thresh > 0).astype(f32)        # heaviside forward
    sums   = segment_sum(spike, binned_degree)   # [20, 128]
    counts = bincount(binned_degree)             # [20]
    v_th_new = where(counts>0, 0.2*sums/max(counts,1) + 0.8*v_threshold,
                     v_threshold)

Returns (spike, v_th_new) like the reference.

Strategy: shard nodes across 8 NeuronCores (62500 nodes each, padded to
63488 = 496 tiles of 128). Per core the device computes spike and the
per-bin partial sums (as sumsT [128 s, 20 b] in PSUM); the tiny [20,128]
cross-core reduction + EMA update run on host.

Three program variants, picked from the actual v_threshold values:
  - uniform: all 20 threshold rows identical (the module's init state) ->
    thresh is bin-independent; compare against one broadcast row, no gather.
  - bf16 gather: rows differ but are bf16-exact -> batched one-hot matmul
    thresh[128,512] = onehotT[80,128].T @ blockdiag(vth)[80,512] in bf16.
  - f32 gather: same with f32 operands (exact for any f32 table).

spike is computed/stored as fp8e4m3 (exact for {0,1}), widened on host.
dv and spike live in DRAM pre-permuted as [128, TILES*128] (partition-
contiguous rows -> large DMA descriptors); host does the permutes.
Per-bin sums: per tile, matmul sumsT[128,20] += spike[128,128].T @ onehot
[128,20] (fp8, fast-weight-load) accumulated in one PSUM bank.
"""

import os
import numpy as np

import concourse.bass as bass
import concourse.bacc as bacc
import concourse.mybir as mybir
from concourse.tile import TileContext
from concourse import bass_utils

F32 = mybir.dt.float32
BF16 = mybir.dt.bfloat16
F8 = mybir.dt.float8e4
ALU = mybir.AluOpType
NP_BF16 = mybir.dt.np(BF16)
NP_F8 = mybir.dt.np(F8)

# Problem geometry (hardcoded per contract).
N_FULL = 500000
S = 128
BINS = 20
CORES = 8
SHARD = N_FULL // CORES          # 62500 real nodes per core
TILES = 496                      # node tiles of 128 per core (padded)
PAD_ROWS = TILES * 128           # 63488 rows fed to each core
GROUP = 4                        # tiles per compare group (free dim 512)
NGROUPS = TILES // GROUP         # 124
CHUNK = 16                       # tiles per DMA chunk (1 MiB of dv)
NCHUNKS = TILES // CHUNK         # 31
GROUPS_PER_CHUNK = CHUNK // GROUP
GAMMA = np.float32(0.2)

_PROGRAM_CACHE = {}
LAST_RESULTS = None              # BassKernelResults of the most recent run


def _build_program(inv_tau: float, mode: str):
    """mode: 'uniform' | 'bf16' | 'f32'"""
    GDT = F32 if mode == "f32" else BF16
    nc = bacc.Bacc(
        "TRN2",
        target_bir_lowering=False,
        debug=False,
        num_devices=CORES,
    )

    dv = nc.dram_tensor("dv", [128, TILES * S], F32, kind="ExternalInput").ap()
    oh = nc.dram_tensor("oh", [128, TILES * BINS], F8, kind="ExternalInput").ap()
    if mode == "uniform":
        vthb = nc.dram_tensor(
            "vthb", [128, GROUP * S], F32, kind="ExternalInput"
        ).ap()
    else:
        ohT = nc.dram_tensor(
            "ohT", [GROUP * BINS, NGROUPS * 128], GDT, kind="ExternalInput"
        ).ap()
        vth4 = nc.dram_tensor(
            "vth4", [GROUP * BINS, GROUP * S], GDT, kind="ExternalInput"
        ).ap()
    spike = nc.dram_tensor(
        "spike", [128, TILES * S], F8, kind="ExternalOutput"
    ).ap()
    sumsT = nc.dram_tensor("sumsT", [S, BINS], F32, kind="ExternalOutput").ap()

    with TileContext(nc) as tc:
        with (
            tc.tile_pool(name="const", bufs=1) as cpool,
            tc.tile_pool(name="io", bufs=4) as iopool,
            tc.tile_pool(name="psum", bufs=4, space="PSUM") as ppool,
            tc.tile_pool(name="acc", bufs=1, space="PSUM") as apool,
        ):
            oh_sb = cpool.tile([128, TILES * BINS], F8)
            nc.sync.dma_start(out=oh_sb[:], in_=oh)
            if mode == "uniform":
                vthb_sb = cpool.tile([128, GROUP * S], F32)
                nc.sync.dma_start(out=vthb_sb[:], in_=vthb)
            else:
                ohT_sb = cpool.tile([GROUP * BINS, NGROUPS * 128], GDT)
                nc.sync.dma_start(out=ohT_sb[:], in_=ohT)
                vth4_sb = cpool.tile([GROUP * BINS, GROUP * S], GDT)
                nc.sync.dma_start(out=vth4_sb[:], in_=vth4)

            sumsT_ps = apool.tile([S, BINS], F32)

            for c in range(NCHUNKS):
                dv_sb = iopool.tile([128, CHUNK * S], F32, tag="dv")
                nc.sync.dma_start(
                    out=dv_sb[:], in_=dv[:, c * CHUNK * S : (c + 1) * CHUNK * S]
                )
                spike_sb = iopool.tile([128, CHUNK * S], F8, tag="spike")

                for g in range(GROUPS_PER_CHUNK):
                    gg = c * GROUPS_PER_CHUNK + g        # global group index
                    lo, hi = g * GROUP * S, (g + 1) * GROUP * S
                    if mode == "uniform":
                        thr = vthb_sb[:]
                    else:
                        th_ps = ppool.tile([128, GROUP * S], F32, tag="th")
                        nc.tensor.matmul(
                            th_ps[:],
                            ohT_sb[:, gg * 128 : (gg + 1) * 128],
                            vth4_sb[:],
                            start=True,
                            stop=True,
                        )
                        thr = th_ps[:]
                    nc.vector.scalar_tensor_tensor(
                        out=spike_sb[:, lo:hi],
                        in0=dv_sb[:, lo:hi],
                        scalar=float(inv_tau),
                        in1=thr,
                        op0=ALU.mult,
                        op1=ALU.is_gt,
                    )
                    for t in range(GROUP):
                        gt = gg * GROUP + t              # global tile index
                        nc.tensor.matmul(
                            sumsT_ps[:],
                            spike_sb[:, (g * GROUP + t) * S : (g * GROUP + t + 1) * S],
                            oh_sb[:, gt * BINS : (gt + 1) * BINS],
                            start=(gt == 0),
                            stop=(gt == TILES - 1),
                            skip_group_check=True,
                        )

                nc.scalar.dma_start(
                    out=spike[:, c * CHUNK * S : (c + 1) * CHUNK * S],
                    in_=spike_sb[:],
                )

            sumsT_sb = cpool.tile([S, BINS], F32)
            nc.scalar.copy(out=sumsT_sb[:], in_=sumsT_ps[:])
            nc.sync.dma_start(out=sumsT, in_=sumsT_sb[:])

    nc.compile()
    return nc


def _pick_mode(vth: np.ndarray) -> str:
    if bool((vth == vth[0:1]).all()):
        return "uniform"
    if bool((vth.astype(NP_BF16).astype(np.float32) == vth).all()):
        return "bf16"
    return "f32"


def _host_inputs(dv, binned_degree, v_threshold, mode: str):
    """Build the 8 per-core input maps (permuted shards + one-hots)."""
    dv = np.ascontiguousarray(dv, dtype=np.float32)
    bins = np.asarray(binned_degree).astype(np.int64)
    vth = np.asarray(v_threshold, dtype=np.float32)
    gdt = np.float32 if mode == "f32" else NP_BF16

    if mode == "uniform":
        vthb = np.ascontiguousarray(
            np.tile(vth[0], (128, GROUP))
        ).astype(np.float32)                             # [128, GROUP*S]
    else:
        vth4 = np.zeros((GROUP * BINS, GROUP * S), dtype=np.float32)
        for t in range(GROUP):
            vth4[t * BINS : (t + 1) * BINS, t * S : (t + 1) * S] = vth
        vth4 = vth4.astype(gdt)

    in_maps = []
    for c in range(CORES):
        start = c * SHARD
        stop = start + PAD_ROWS
        if stop <= N_FULL:
            shard = dv[start:stop]
        else:
            shard = np.zeros((PAD_ROWS, S), dtype=np.float32)
            shard[: N_FULL - start] = dv[start:]
        # permute to [partition, tile*S]: dv2[p, t*S+s] = shard[t*128+p, s]
        dv2 = np.ascontiguousarray(
            shard.reshape(TILES, 128, S).transpose(1, 0, 2)
        ).reshape(128, TILES * S)

        b = np.full(PAD_ROWS, -1, dtype=np.int64)
        b[:SHARD] = bins[start : start + SHARD]          # only own rows counted
        binT = b.reshape(TILES, 128).T                   # [128, TILES]
        onehot = binT[:, :, None] == np.arange(BINS)     # [128, TILES, 20] bool
        oh = np.ascontiguousarray(
            onehot.reshape(128, TILES * BINS).astype(NP_F8)
        )
        m = {"dv": dv2, "oh": oh}
        if mode == "uniform":
            m["vthb"] = vthb
        else:
            m["ohT"] = np.ascontiguousarray(
                onehot.reshape(128, NGROUPS, GROUP, BINS)
                .transpose(2, 3, 1, 0)                   # [GROUP, BINS, ngrp, 128]
                .reshape(GROUP * BINS, NGROUPS * 128)
                .astype(gdt)
            )
            m["vth4"] = vth4
        in_maps.append(m)
    return in_maps


def kernel(dv, binned_degree, v_threshold, tau, alpha):
    global LAST_RESULTS
    inv_tau = 1.0 / float(tau)

    vth = np.asarray(v_threshold, dtype=np.float32)
    mode = _pick_mode(vth)

    key = (inv_tau, mode)
    if key not in _PROGRAM_CACHE:
        _PROGRAM_CACHE[key] = _build_program(inv_tau, mode)
    nc = _PROGRAM_CACHE[key]

    in_maps = _host_inputs(dv, binned_degree, v_threshold, mode)

    trace = bool(int(os.environ.get("KERNEL_TRACE", "0")))
    res = bass_utils.run_bass_kernel_spmd(
        nc, in_maps, core_ids=list(range(CORES)), trace=trace
    )
    LAST_RESULTS = res

    spike = np.empty((N_FULL, S), dtype=np.float32)
    for c in range(CORES):
        sp2 = res.results[c]["spike"]                    # [128, TILES*S] fp8
        sp = sp2.reshape(128, TILES, S).transpose(1, 0, 2).reshape(PAD_ROWS, S)
        spike[c * SHARD : (c + 1) * SHARD] = sp[:SHARD]  # widen fp8 -> f32

    sums = np.zeros((BINS, S), dtype=np.float32)
    for c in range(CORES):
        sums += res.results[c]["sumsT"].T

    bins = np.asarray(binned_degree).astype(np.int64)
    counts = np.bincount(bins, minlength=BINS).astype(np.float32)
    mean = sums / np.maximum(counts, np.float32(1.0))[:, None]
    v_th_new = np.where(
        (counts > 0)[:, None],
        GAMMA * mean + (np.float32(1.0) - GAMMA) * vth,
        vth,
    ).astype(np.float32)

    return spike, v_th_new


# revision 5
# speedup vs baseline: 2.6873x; 1.1995x over previous
"""Trainium2 Bass kernel for BaseNeuron degree-feature spiking forward.

Computes, for dv [500000, 128] f32, binned_degree [500000] i32,
v_threshold [20, 128] f32, tau, alpha scalars:

    v      = dv / tau
    thresh = v_threshold[binned_degree]          # per-node row gather
    spike  = (v - thresh > 0).astype(f32)        # heaviside forward
    sums   = segment_sum(spike, binned_degree)   # [20, 128]
    counts = bincount(binned_degree)             # [20]
    v_th_new = where(counts>0, 0.2*sums/max(counts,1) + 0.8*v_threshold,
                     v_threshold)

Returns (spike, v_th_new) like the reference.

Strategy: shard nodes across 8 NeuronCores (62500 nodes each, padded to
63488 = 496 tiles of 128). Per core the device computes spike and the
per-bin partial sums (as sumsT [128 s, 20 b] in PSUM); the tiny [20,128]
cross-core reduction + EMA update run on host.

Three program variants, picked from the actual v_threshold values:
  - uniform: all 20 threshold rows identical (the module's init state) ->
    thresh is bin-independent; compare against one broadcast row, no gather.
  - bf16 gather: rows differ but are bf16-exact -> batched one-hot matmul
    thresh[128,512] = onehotT[80,128].T @ blockdiag(vth)[80,512] in bf16.
  - f32 gather: same with f32 operands (exact for any f32 table).

spike is computed/stored as fp8e4m3 (exact for {0,1}), widened on host.
dv and spike live in DRAM pre-permuted as [128, TILES*128] (partition-
contiguous rows -> large DMA descriptors); host does the permutes.
Per-bin sums: per tile, matmul sumsT[128,20] += spike[128,128].T @ onehot
[128,20] (fp8, fast-weight-load) accumulated in one PSUM bank.
"""

import os
import numpy as np

import concourse.bass as bass
import concourse.bacc as bacc
import concourse.mybir as mybir
from concourse.tile import TileContext
from concourse import bass_utils

F32 = mybir.dt.float32
BF16 = mybir.dt.bfloat16
F8 = mybir.dt.float8e4
ALU = mybir.AluOpType
NP_BF16 = mybir.dt.np(BF16)
NP_F8 = mybir.dt.np(F8)

# Problem geometry (hardcoded per contract).
N_FULL = 500000
S = 128
BINS = 20
CORES = 8
SHARD = N_FULL // CORES          # 62500 real nodes per core
TILES = 496                      # node tiles of 128 per core (padded)
PAD_ROWS = TILES * 128           # 63488 rows fed to each core
GROUP = 4                        # tiles per compare group (free dim 512)
NGROUPS = TILES // GROUP         # 124
CHUNK = 16                       # tiles per DMA chunk (1 MiB of dv)
NCHUNKS = TILES // CHUNK         # 31
GROUPS_PER_CHUNK = CHUNK // GROUP
GAMMA = np.float32(0.2)

_PROGRAM_CACHE = {}
LAST_RESULTS = None              # BassKernelResults of the most recent run


def _build_program(inv_tau: float, mode: str):
    """mode: 'uniform' | 'bf16' | 'f32'"""
    GDT = F32 if mode == "f32" else BF16
    nc = bacc.Bacc(
        "TRN2",
        target_bir_lowering=False,
        debug=False,
        num_devices=CORES,
    )

    dv = nc.dram_tensor("dv", [128, TILES * S], F32, kind="ExternalInput").ap()
    oh = nc.dram_tensor("oh", [128, TILES * BINS], F8, kind="ExternalInput").ap()
    if mode == "uniform":
        vthb = nc.dram_tensor(
            "vthb", [128, GROUP * S], F32, kind="ExternalInput"
        ).ap()
    else:
        ohT = nc.dram_tensor(
            "ohT", [GROUP * BINS, NGROUPS * 128], GDT, kind="ExternalInput"
        ).ap()
        vth4 = nc.dram_tensor(
            "vth4", [GROUP * BINS, GROUP * S], GDT, kind="ExternalInput"
        ).ap()
    spike = nc.dram_tensor(
        "spike", [128, TILES * S], F8, kind="ExternalOutput"
    ).ap()
    sumsT = nc.dram_tensor("sumsT", [S, BINS], F32, kind="ExternalOutput").ap()

    with TileContext(nc) as tc:
        with (
            tc.tile_pool(name="const", bufs=1) as cpool,
            tc.tile_pool(name="io", bufs=8) as iopool,
            tc.tile_pool(name="psum", bufs=4, space="PSUM") as ppool,
            tc.tile_pool(name="acc", bufs=1, space="PSUM") as apool,
        ):
            # Constants ride the scalar-engine HWDGE ring so the first dv
            # chunk can start immediately on the sync ring.
            oh_sb = cpool.tile([128, TILES * BINS], F8)
            nc.scalar.dma_start(out=oh_sb[:], in_=oh)
            if mode == "uniform":
                vthb_sb = cpool.tile([128, GROUP * S], F32)
                nc.scalar.dma_start(out=vthb_sb[:], in_=vthb)
            else:
                ohT_sb = cpool.tile([GROUP * BINS, NGROUPS * 128], GDT)
                nc.scalar.dma_start(out=ohT_sb[:], in_=ohT)
                vth4_sb = cpool.tile([GROUP * BINS, GROUP * S], GDT)
                nc.scalar.dma_start(out=vth4_sb[:], in_=vth4)

            sumsT_ps = apool.tile([S, BINS], F32)

            for c in range(NCHUNKS):
                dv_sb = iopool.tile([128, CHUNK * S], F32, tag="dv")
                nc.sync.dma_start(
                    out=dv_sb[:], in_=dv[:, c * CHUNK * S : (c + 1) * CHUNK * S]
                )
                spike_sb = iopool.tile([128, CHUNK * S], F8, tag="spike")

                for g in range(GROUPS_PER_CHUNK):
                    gg = c * GROUPS_PER_CHUNK + g        # global group index
                    lo, hi = g * GROUP * S, (g + 1) * GROUP * S
                    if mode == "uniform":
                        thr = vthb_sb[:]
                    else:
                        th_ps = ppool.tile([128, GROUP * S], F32, tag="th")
                        nc.tensor.matmul(
                            th_ps[:],
                            ohT_sb[:, gg * 128 : (gg + 1) * 128],
                            vth4_sb[:],
                            start=True,
                            stop=True,
                        )
                        thr = th_ps[:]
                    nc.vector.scalar_tensor_tensor(
                        out=spike_sb[:, lo:hi],
                        in0=dv_sb[:, lo:hi],
                        scalar=float(inv_tau),
                        in1=thr,
                        op0=ALU.mult,
                        op1=ALU.is_gt,
                    )
                    for t in range(GROUP):
                        gt = gg * GROUP + t              # global tile index
                        nc.tensor.matmul(
                            sumsT_ps[:],
                            spike_sb[:, (g * GROUP + t) * S : (g * GROUP + t + 1) * S],
                            oh_sb[:, gt * BINS : (gt + 1) * BINS],
                            start=(gt == 0),
                            stop=(gt == TILES - 1),
                            skip_group_check=True,
                        )

                nc.scalar.dma_start(
                    out=spike[:, c * CHUNK * S : (c + 1) * CHUNK * S],
                    in_=spike_sb[:],
                )

            sumsT_sb = cpool.tile([S, BINS], F32)
            nc.scalar.copy(out=sumsT_sb[:], in_=sumsT_ps[:])
            nc.sync.dma_start(out=sumsT, in_=sumsT_sb[:])

    nc.compile()
    return nc


def _pick_mode(vth: np.ndarray) -> str:
    if bool((vth == vth[0:1]).all()):
        return "uniform"
    if bool((vth.astype(NP_BF16).astype(np.float32) == vth).all()):
        return "bf16"
    return "f32"


def _host_inputs(dv, binned_degree, v_threshold, mode: str):
    """Build the 8 per-core input maps (permuted shards + one-hots)."""
    dv = np.ascontiguousarray(dv, dtype=np.float32)
    bins = np.asarray(binned_degree).astype(np.int64)
    vth = np.asarray(v_threshold, dtype=np.float32)
    gdt = np.float32 if mode == "f32" else NP_BF16

    if mode == "uniform":
        vthb = np.ascontiguousarray(
            np.tile(vth[0], (128, GROUP))
        ).astype(np.float32)                             # [128, GROUP*S]
    else:
        vth4 = np.zeros((GROUP * BINS, GROUP * S), dtype=np.float32)
        for t in range(GROUP):
            vth4[t * BINS : (t + 1) * BINS, t * S : (t + 1) * S] = vth
        vth4 = vth4.astype(gdt)

    in_maps = []
    for c in range(CORES):
        start = c * SHARD
        stop = start + PAD_ROWS
        if stop <= N_FULL:
            shard = dv[start:stop]
        else:
            shard = np.zeros((PAD_ROWS, S), dtype=np.float32)
            shard[: N_FULL - start] = dv[start:]
        # permute to [partition, tile*S]: dv2[p, t*S+s] = shard[t*128+p, s]
        dv2 = np.ascontiguousarray(
            shard.reshape(TILES, 128, S).transpose(1, 0, 2)
        ).reshape(128, TILES * S)

        b = np.full(PAD_ROWS, -1, dtype=np.int64)
        b[:SHARD] = bins[start : start + SHARD]          # only own rows counted
        binT = b.reshape(TILES, 128).T                   # [128, TILES]
        onehot = binT[:, :, None] == np.arange(BINS)     # [128, TILES, 20] bool
        oh = np.ascontiguousarray(
            onehot.reshape(128, TILES * BINS).astype(NP_F8)
        )
        m = {"dv": dv2, "oh": oh}
        if mode == "uniform":
            m["vthb"] = vthb
        else:
            m["ohT"] = np.ascontiguousarray(
                onehot.reshape(128, NGROUPS, GROUP, BINS)
                .transpose(2, 3, 1, 0)                   # [GROUP, BINS, ngrp, 128]
                .reshape(GROUP * BINS, NGROUPS * 128)
                .astype(gdt)
            )
            m["vth4"] = vth4
        in_maps.append(m)
    return in_maps


def kernel(dv, binned_degree, v_threshold, tau, alpha):
    global LAST_RESULTS
    inv_tau = 1.0 / float(tau)

    vth = np.asarray(v_threshold, dtype=np.float32)
    mode = _pick_mode(vth)

    key = (inv_tau, mode)
    if key not in _PROGRAM_CACHE:
        _PROGRAM_CACHE[key] = _build_program(inv_tau, mode)
    nc = _PROGRAM_CACHE[key]

    in_maps = _host_inputs(dv, binned_degree, v_threshold, mode)

    trace = bool(int(os.environ.get("KERNEL_TRACE", "0")))
    res = bass_utils.run_bass_kernel_spmd(
        nc, in_maps, core_ids=list(range(CORES)), trace=trace
    )
    LAST_RESULTS = res

    spike = np.empty((N_FULL, S), dtype=np.float32)
    for c in range(CORES):
        sp2 = res.results[c]["spike"]                    # [128, TILES*S] fp8
        sp = sp2.reshape(128, TILES, S).transpose(1, 0, 2).reshape(PAD_ROWS, S)
        spike[c * SHARD : (c + 1) * SHARD] = sp[:SHARD]  # widen fp8 -> f32

    sums = np.zeros((BINS, S), dtype=np.float32)
    for c in range(CORES):
        sums += res.results[c]["sumsT"].T

    bins = np.asarray(binned_degree).astype(np.int64)
    counts = np.bincount(bins, minlength=BINS).astype(np.float32)
    mean = sums / np.maximum(counts, np.float32(1.0))[:, None]
    v_th_new = np.where(
        (counts > 0)[:, None],
        GAMMA * mean + (np.float32(1.0) - GAMMA) * vth,
        vth,
    ).astype(np.float32)

    return spike, v_th_new
